# revision 12
# baseline (speedup 1.0000x reference)
"""Trainium2 Bass kernel for the LoRA-mixture layer.

Math (derived from the reference's interleave):  for batch b,
  y[b] = relu( 0.25 * x[b] @ Bcat_b @ Acat_b )
where Bcat_b = concat of adapter_b[4b:4b+4] along rank (rank 16),
      Acat_b = concat of adapter_a[4b:4b+4] along rank.

Sharding: data-parallel, batch b -> core b (8 batches, 8 cores).

Per-core dataflow (x_i is [4096, 2048] f32):
  for each s-slab of 512 rows:
    DMA in x slab [128p, 4t, 2048d]
    PE-transpose 128x128 blocks -> xT chunks [128d, 512s]  (fp32, exact)
    ACT-evict PSUM->SBUF (rounds to mm dtype)
    mm1: hT[16, 512] += BcatChunk[128,16].T @ xTchunk[128,512]  (16 chunks)
    ACT-evict hT
    mm2: y[128,512] = hTslice[16,128].T @ AcatSlice[16,512]
    DVE relu-evict PSUM->SBUF  (0.25 scale pre-folded into Acat on host)
    DMA out y slab
"""

import numpy as np

import concourse.bass as bass
import concourse.mybir as mybir
import concourse.tile as tile
from concourse import bacc
from concourse.bass_utils import run_bass_kernel_spmd
from concourse.masks import make_identity

B, S, D = 8, 4096, 2048
R = 16               # concatenated rank per batch (4 adapters x rank 4)
N_CORES = 8
SLAB = 256           # s rows per slab
NSLAB = S // SLAB    # 16
TS = SLAB // 128     # 2 s-subtiles per slab
DC = D // 128        # 16 contraction chunks
CG = 4               # transpose-chunk group size (chunks per PSUM evict)
NDP = D // 512       # 4 output-column chunks

F32 = mybir.dt.float32
F32R = mybir.dt.float32r

MM_DT = F32R         # dtype used for the two matmuls (stationary+moving)
TR_F32R = True       # run PE transposes in f32r (1.5 vs 2 cyc/row)


def build_nc():
    nc = bacc.Bacc("TRN2", target_bir_lowering=False, debug=False)

    x = nc.dram_tensor("x", [S, D], F32R if TR_F32R else F32, kind="ExternalInput")
    bcat = nc.dram_tensor("bcat", [D, R], F32, kind="ExternalInput")
    acat = nc.dram_tensor("acat", [R, D], F32, kind="ExternalInput")
    y = nc.dram_tensor("y", [S, D], F32, kind="ExternalOutput")

    with tile.TileContext(nc) as tc:
        with (
            tc.tile_pool(name="const", bufs=1) as cpool,
            tc.tile_pool(name="xin", bufs=3) as xin_pool,
            tc.tile_pool(name="xt", bufs=8) as xt_pool,
            tc.tile_pool(name="ht", bufs=2) as ht_pool,
            tc.tile_pool(name="yout", bufs=3) as y_pool,
            tc.tile_pool(name="pt", bufs=2, space="PSUM") as pt_pool,
            tc.tile_pool(name="ph", bufs=1, space="PSUM") as ph_pool,
            tc.tile_pool(name="py", bufs=3, space="PSUM") as py_pool,
        ):
            ident_f32 = cpool.tile([128, 128], F32)
            make_identity(nc, ident_f32[:])
            if TR_F32R:
                ident = cpool.tile([128, 128], F32R)
                nc.scalar.copy(ident[:], ident_f32[:])
            else:
                ident = ident_f32

            # Adapter factors: DMA f32 staging, then round into MM_DT tiles.
            # Bcat [D, R] -> SBUF [128, DC, R]; chunk c = Bcat[c*128:(c+1)*128, :]
            bcat_st = cpool.tile([128, DC, R], F32)
            nc.sync.dma_start(
                out=bcat_st[:], in_=bcat.ap().rearrange("(c p) r -> p c r", p=128)
            )
            # Acat [R, D] (pre-scaled by 0.25 on host)
            acat_st = cpool.tile([R, D], F32)
            nc.sync.dma_start(out=acat_st[:], in_=acat.ap())
            if MM_DT is F32:
                bcat_sb, acat_sb = bcat_st, acat_st
            else:
                bcat_sb = cpool.tile([128, DC, R], MM_DT)
                nc.scalar.copy(bcat_sb[:], bcat_st[:])
                acat_sb = cpool.tile([R, D], MM_DT)
                nc.scalar.copy(acat_sb[:], acat_st[:])

            x_ap = x.ap().rearrange("(i t p) d -> i p t d", p=128, t=TS)
            y_ap = y.ap().rearrange("(i t p) d -> i p t d", p=128, t=TS)

            for i in range(NSLAB):
                x_sb = xin_pool.tile([128, TS, D], F32R if TR_F32R else F32)
                nc.sync.dma_start(out=x_sb[:], in_=x_ap[i])

                # transpose x slab into chunk groups [128 d, CG, SLAB s]
                xt_groups = []
                for g in range(DC // CG):
                    # 2 PSUM banks: CG chunks x TS subtiles of 128x128
                    pt = pt_pool.tile(
                        [128, CG, TS, 128], F32R if TR_F32R else F32, tag="pt"
                    )
                    for cc in range(CG):
                        c = g * CG + cc
                        for t in range(TS):
                            nc.tensor.transpose(
                                pt[:, cc, t, :],
                                x_sb[:, t, c * 128 : (c + 1) * 128],
                                ident[:],
                            )
                    xt_sb = xt_pool.tile([128, CG, SLAB], MM_DT, tag="xt")
                    nc.scalar.copy(xt_sb[:], pt[:])
                    xt_groups.append(xt_sb)

                # mm1: hT [R, SLAB] accumulated over DC chunks
                ht_ps = ph_pool.tile([R, SLAB], F32, tag="ph")
                for c in range(DC):
                    nc.tensor.matmul(
                        ht_ps[:],
                        bcat_sb[:, c, :],
                        xt_groups[c // CG][:, c % CG, :],
                        start=(c == 0),
                        stop=(c == DC - 1),
                    )
                ht_sb = ht_pool.tile([R, SLAB], MM_DT, tag="ht")
                nc.scalar.copy(ht_sb[:], ht_ps[:])

                # mm2 + relu eviction
                y_sb = y_pool.tile([128, TS, D], F32)
                for t in range(TS):
                    for dp in range(NDP):
                        py = py_pool.tile([128, 512], F32, tag="py")
                        nc.tensor.matmul(
                            py[:],
                            ht_sb[:, t * 128 : (t + 1) * 128],
                            acat_sb[:, dp * 512 : (dp + 1) * 512],
                            start=True,
                            stop=True,
                        )
                        nc.vector.tensor_scalar_max(
                            y_sb[:, t, dp * 512 : (dp + 1) * 512], py[:], 0.0
                        )
                nc.sync.dma_start(out=y_ap[i], in_=y_sb[:])

    nc.compile()
    return nc


_NC = None


def _get_nc():
    global _NC
    if _NC is None:
        _NC = build_nc()
    return _NC


def make_in_maps(x, adapter_b, adapter_a):
    in_maps = []
    for b in range(B):
        bc = np.ascontiguousarray(
            adapter_b[4 * b : 4 * b + 4].transpose(1, 0, 2).reshape(D, R)
        ).astype(np.float32)
        ac = np.ascontiguousarray(
            adapter_a[4 * b : 4 * b + 4].reshape(R, D) * 0.25
        ).astype(np.float32)
        in_maps.append(
            {
                "x": np.ascontiguousarray(x[b]).astype(np.float32),
                "bcat": bc,
                "acat": ac,
            }
        )
    return in_maps


def run(x, adapter_b, adapter_a, **run_kwargs):
    nc = _get_nc()
    in_maps = make_in_maps(x, adapter_b, adapter_a)
    res = run_bass_kernel_spmd(nc, in_maps, list(range(N_CORES)), **run_kwargs)
    out = np.stack([res.results[i]["y"] for i in range(N_CORES)])
    return out, res


def kernel(x, adapter_b, adapter_a):
    out, _ = run(x, adapter_b, adapter_a)
    return out


# revision 14
# speedup vs baseline: 1.2185x; 1.2185x over previous
"""Trainium2 Bass kernel for the LoRA-mixture layer.

Math (derived from the reference's interleave):  for batch b,
  y[b] = relu( 0.25 * x[b] @ Bcat_b @ Acat_b )
where Bcat_b = concat of adapter_b[4b:4b+4] along rank (rank 16),
      Acat_b = concat of adapter_a[4b:4b+4] along rank.

Sharding: data-parallel, batch b -> core b (8 batches, 8 cores).

Per-core dataflow (x_i is [4096, 2048] f32):
  for each s-slab of 512 rows:
    DMA in x slab [128p, 4t, 2048d]
    PE-transpose 128x128 blocks -> xT chunks [128d, 512s] (fp32, exact)
    ACT-evict PSUM->SBUF, rounding to f32r
    mm1: hT4[128, 512] += bcat4Chunk[128,128].T @ xTchunk[128,512]
         where bcat4 has Bcat replicated at column offsets 0/32/64/96
         -> hT lands replicated at partition offsets 0/32/64/96
    ACT-evict hT4 (one op)
    mm2: 4 concurrent row-group matmuls (tile_position) per d'-chunk:
         y[128,512] = hT[16,128].T @ Acat[16,512]
    DVE relu-evict PSUM->SBUF (0.25 folded into Acat on host)
    DMA out y slab
"""

import numpy as np

import concourse.bass as bass
import concourse.mybir as mybir
import concourse.tile as tile
from concourse import bacc
from concourse.bass_utils import run_bass_kernel_spmd
from concourse.masks import make_identity

B, S, D = 8, 4096, 2048
R = 16               # concatenated rank per batch (4 adapters x rank 4)
N_CORES = 8
SLAB = 512           # s rows per slab
NSLAB = S // SLAB    # 8
TS = SLAB // 128     # 4 s-subtiles per slab
DC = D // 128        # 16 contraction chunks
NDP = D // 512       # 4 output-column chunks

F32 = mybir.dt.float32
F32R = mybir.dt.float32r


def build_nc():
    nc = bacc.Bacc("TRN2", target_bir_lowering=False, debug=False)

    x = nc.dram_tensor("x", [S, D], F32, kind="ExternalInput")
    # bcat4 [D, 128]: Bcat columns replicated at offsets 0/32/64/96 (zeros
    # elsewhere) so mm1 emits hT at 4 partition offsets for row-packed mm2.
    bcat4 = nc.dram_tensor("bcat4", [D, 128], F32R, kind="ExternalInput")
    acat = nc.dram_tensor("acat", [R, D], F32R, kind="ExternalInput")
    y = nc.dram_tensor("y", [S, D], F32, kind="ExternalOutput")

    with tile.TileContext(nc) as tc:
        with (
            tc.tile_pool(name="const", bufs=1) as cpool,
            tc.tile_pool(name="xin", bufs=2) as xin_pool,
            tc.tile_pool(name="xt", bufs=20) as xt_pool,
            tc.tile_pool(name="ht", bufs=2) as ht_pool,
            tc.tile_pool(name="yout", bufs=2) as y_pool,
            tc.tile_pool(name="pt", bufs=2, space="PSUM") as pt_pool,
            tc.tile_pool(name="ph", bufs=2, space="PSUM") as ph_pool,
            tc.tile_pool(name="py", bufs=4, space="PSUM") as py_pool,
        ):
            ident = cpool.tile([128, 128], F32)
            make_identity(nc, ident[:])

            # bcat4 [D, 128] -> SBUF [128, DC, 128]
            bcat_sb = cpool.tile([128, DC, 128], F32R)
            nc.sync.dma_start(
                out=bcat_sb[:], in_=bcat4.ap().rearrange("(c p) r -> p c r", p=128)
            )
            # Acat replicated at partition offsets 0/32/64/96 for row-packed
            # mm2 (rhs partitions must match the row group). Unwritten rows
            # are never read.
            acat_rep = cpool.tile([128, D], F32R)
            for j in range(4):
                nc.sync.dma_start(
                    out=acat_rep[32 * j : 32 * j + R, :], in_=acat.ap()
                )

            x_ap = x.ap().rearrange("(i t p) d -> i p t d", p=128, t=TS)
            y_ap = y.ap().rearrange("(i t p) d -> i p t d", p=128, t=TS)

            for i in range(NSLAB):
                x_sb = xin_pool.tile([128, TS, D], F32)
                nc.sync.dma_start(out=x_sb[:], in_=x_ap[i])

                # transpose x slab into DC chunks of [128 d, SLAB s]
                xt_chunks = []
                for c in range(DC):
                    pt = pt_pool.tile([128, TS, 128], F32, tag="pt")
                    for t in range(TS):
                        nc.tensor.transpose(
                            pt[:, t, :],
                            x_sb[:, t, c * 128 : (c + 1) * 128],
                            ident[:],
                        )
                    xt_sb = xt_pool.tile([128, SLAB], F32R, tag="xt")
                    nc.scalar.copy(xt_sb[:], pt[:])
                    xt_chunks.append(xt_sb)

                # mm1: hT4 [128, SLAB]: hT replicated at partitions 0/32/64/96
                ht_ps = ph_pool.tile([128, SLAB], F32, tag="ph")
                for c in range(DC):
                    nc.tensor.matmul(
                        ht_ps[:],
                        bcat_sb[:, c, :],
                        xt_chunks[c][:],
                        start=(c == 0),
                        stop=(c == DC - 1),
                    )
                ht_rep = ht_pool.tile([128, SLAB], F32R, tag="ht")
                nc.scalar.copy(ht_rep[:], ht_ps[:])

                # mm2: per d'-chunk, 4 concurrent row-group matmuls (t)
                y_sb = y_pool.tile([128, TS, D], F32)
                for dp in range(NDP):
                    pys = []
                    for t in range(TS):
                        py = py_pool.tile([128, 512], F32, tag="py")
                        nc.tensor.matmul(
                            py[:],
                            ht_rep[32 * t : 32 * t + R, t * 128 : (t + 1) * 128],
                            acat_rep[32 * t : 32 * t + R, dp * 512 : (dp + 1) * 512],
                            start=True,
                            stop=True,
                            tile_position=(32 * t, 0),
                        )
                        pys.append(py)
                    for t in range(TS):
                        nc.vector.tensor_scalar_max(
                            y_sb[:, t, dp * 512 : (dp + 1) * 512], pys[t][:], 0.0
                        )
                nc.sync.dma_start(out=y_ap[i], in_=y_sb[:])

    nc.compile()
    return nc


_NC = None


def _get_nc():
    global _NC
    if _NC is None:
        _NC = build_nc()
    return _NC


def make_in_maps(x, adapter_b, adapter_a):
    in_maps = []
    for b in range(B):
        bc = np.ascontiguousarray(
            adapter_b[4 * b : 4 * b + 4].transpose(1, 0, 2).reshape(D, R)
        ).astype(np.float32)
        bc4 = np.zeros((D, 128), dtype=np.float32)
        for j in range(4):
            bc4[:, 32 * j : 32 * j + R] = bc
        ac = np.ascontiguousarray(
            adapter_a[4 * b : 4 * b + 4].reshape(R, D) * 0.25
        ).astype(np.float32)
        in_maps.append(
            {
                "x": np.ascontiguousarray(x[b]).astype(np.float32),
                "bcat4": bc4,
                "acat": ac,
            }
        )
    return in_maps


def run(x, adapter_b, adapter_a, **run_kwargs):
    nc = _get_nc()
    in_maps = make_in_maps(x, adapter_b, adapter_a)
    res = run_bass_kernel_spmd(nc, in_maps, list(range(N_CORES)), **run_kwargs)
    out = np.stack([res.results[i]["y"] for i in range(N_CORES)])
    return out, res


def kernel(x, adapter_b, adapter_a):
    out, _ = run(x, adapter_b, adapter_a)
    return out
